# revision 5
# baseline (speedup 1.0000x reference)
"""MoE QLoRA linear kernel for Trainium2 (8 NeuronCores, data-parallel over tokens).

Computes, for x:(B,S,IN) f32:
    base  = x @ W.T + b
    gates = softmax(x @ Wr.T)                       # (tok, E)
    proj  = x @ A[e].T                              # (tok, E, R)
    out   = base + sum_e SCALE * gates[...,e] * (proj[...,e,:] @ Bm[e].T)

Key algebraic fold: the gated expert mix is a single rank-(E*R) matmul:
    wproj[t, er] = SCALE * gates[t, e] * proj[t, er]          (er = e*R+r)
    lora[t, o]   = sum_er wproj[t, er] * Bcat[er, o]          (Bcat[er,o] = Bm[e,o,r])
and the bias b is folded in as an extra contraction row (wproj row of ones,
Bcat row = b), so base+lora+bias all accumulate in one PSUM group on the PE.

Per-core kernel (1024 tokens), everything oriented (feature-partition, token-free):
  phase 1: PSUM(36,512) = [A;Wr]^T-stationary matmuls over 32 k-tiles ->
           proj rows 0..31, router logits rows 32..35; softmax via exp +
           PE ones-matmul partition reductions/broadcasts; wproj written fp16.
           The o-tile-0 base k-loop is emitted between the proj matmuls and
           the gating chain so the PE stays busy while ACT/DVE run softmax.
  phase 2: for each of 32 o-tiles: out(128o, t) = W-tile-stationary matmul
           over 32 k-tiles + one lora matmul (k=33) accumulated into PSUM,
           copy to SBUF, DMA out as (OUT, tok); host transposes back.

All matmul inputs are fp16 (host-cast; PE runs fp16 at full bf16 rate,
fp32 PSUM accumulation). Host pre-tiles all layouts so every DMA is
contiguous and the kernel needs zero on-chip transposes.

Perf note: this toolchain's walrus runs with --enable-ldw-opt=false (=true
crashes codegen), so every matmul gets its own LDWEIGHTS (~53ns each,
~115us/core) on top of the ~490us cost-model prediction; measured device
time is ~600-650us/core. Matmul count (2182) is at the hardware floor
(m<=128, n<=512/PSUM bank, k<=128), so no tiling change reduces it. The
timeline trace shows the PE sequencer saturated end-to-end; PE-engine idle
is only startup DMA (~11us, both alternate DMA rings measured worse) and
the framework tail drain (~5us).

LDWEIGHTS dedup (landed): after nc.compile(), _dedupe_ldweights deletes the
redundant consecutive InstLdweights (the second load of each same-stationary
(o-tile, k) pair — wait/update-free, so the semaphore graph is untouched).
walrus pairs a standalone InstLdweights with the following non-self-loading
matmuls for 2-byte dtypes, so the surviving load covers both slab matmuls.
"""

import numpy as np

import concourse.bass as bass
import concourse.tile as tile
from concourse import bacc, mybir
from concourse import bass_utils

# Problem shape (hardcoded; kernel.py must be self-contained)
B, S, IN, OUT, E, R = 4, 2048, 4096, 4096, 4, 8
SCALE = 16.0 / 8.0
N_CORES = 8
TOK = B * S                  # 8192 tokens
TPC = TOK // N_CORES         # 1024 tokens per core
P = 128                      # partitions
KT = IN // P                 # 32 k-tiles (contraction)
OT = OUT // P                # 32 output tiles
NSLAB = 512                  # moving-operand free size (PSUM bank = 512 f32)
NS = TPC // NSLAB            # 2 token slabs per core
ER = E * R                   # 32 low-rank rows
ERA = ER + 1                 # +1 ones row (bias fold)

F16 = mybir.dt.float16
F32 = mybir.dt.float32

_NC = None


def _dedupe_ldweights(nc):
    """Delete redundant consecutive InstLdweights from the PE streams.

    With --enable-ldw-opt=false every matmul gets its own ~53ns LDWEIGHTS;
    when consecutive matmuls share a stationary (the two token-slab matmuls
    of each (o-tile, k) pair) the repeat loads are pure PE stalls.  A
    deleted load's matmuls pair with the previous (identical) load.  Only
    wait/update-free loads whose key matches the immediately preceding PE
    ldweights are deleted; any other PE instruction type resets the match.

    CRITICAL: a matmul with an fp32 stationary is self-loading and clobbers
    the PE array (the tile scheduler interleaves the f32 gating matmuls into
    the o-tile-1 k-loop), so any matmul whose stationary key doesn't match
    the last load also resets the match.
    """

    def _wkey(ap):
        return (ap.memref, ap.offset, str(ap.ap), str(ap.dtype))

    ndel = 0
    for b in nc.main_func.blocks:
        il = b.instructions
        last_key = None
        dead_ids = set()
        for inst in il:
            if inst.engine != mybir.EngineType.PE:
                continue
            if isinstance(inst, mybir.InstLdweights):
                ap = inst.ins[0]
                key = (_wkey(ap), str(inst.tile_position), str(inst.tile_size),
                       str(inst.perf_mode), str(inst.is_transpose))
                if key == last_key and not inst.has_wait() and not inst.has_update():
                    dead_ids.add(id(inst))
                else:
                    last_key = key
            elif isinstance(inst, mybir.InstMatmult):
                if last_key is None or _wkey(inst.ins[1]) != last_key[0]:
                    last_key = None
        if dead_ids:
            keep = [i for i in il if id(i) not in dead_ids]
            il.clear()
            il.extend(keep)
            ndel += len(dead_ids)
    return ndel


def build_nc(reps=1, ns=NS):
    NS_ = ns
    nc = bacc.Bacc("TRN2", target_bir_lowering=False, debug=False)

    xd = nc.dram_tensor("xd", [P, KT, TPC], F16, kind="ExternalInput")
    wd = nc.dram_tensor("wd", [OT, P, KT, P], F16, kind="ExternalInput")
    artd = nc.dram_tensor("artd", [P, KT, ER + E], F16, kind="ExternalInput")
    btd = nc.dram_tensor("btd", [ERA, OUT], F16, kind="ExternalInput")
    seld = nc.dram_tensor("seld", [E, ER], F32, kind="ExternalInput")
    od = nc.dram_tensor("od", [OUT, TPC], F32, kind="ExternalOutput")

    with tile.TileContext(nc) as tc:
        with (
            tc.tile_pool(name="consts", bufs=1) as consts,
            tc.tile_pool(name="wpool", bufs=3) as wpool,
            tc.tile_pool(name="opool", bufs=3) as opool,
            tc.tile_pool(name="small", bufs=2) as small,
            tc.tile_pool(name="psum_proj", bufs=1, space="PSUM") as psum_proj,
            tc.tile_pool(name="psum_base", bufs=2, space="PSUM") as psum_base,
        ):
            art_sb = consts.tile([P, KT, ER + E], F16)
            nc.sync.dma_start(out=art_sb[:], in_=artd[:])
            bt_sb = consts.tile([ERA, OUT], F16)
            nc.sync.dma_start(out=bt_sb[:], in_=btd[:])
            sel_sb = consts.tile([E, ER], F32)
            nc.sync.dma_start(out=sel_sb[:], in_=seld[:])

            w_tiles = {}

            def load_w(ot):
                w_sb = wpool.tile([P, KT, P], F16, tag="w", name="w_sb")
                nc.sync.dma_start(out=w_sb[:], in_=wd[ot])
                w_tiles[ot] = w_sb

            # first two W tiles before the bulk x load: o-tile 0 can start
            # as soon as phase-1 finishes on the PE
            load_w(0)
            load_w(1)

            # Resident activations: x^T tiled (p=i%128, k=i//128, t), fp16, 8 MiB.
            x_sb = consts.tile([P, KT, TPC], F16)
            for k in range(KT):
                nc.sync.dma_start(out=x_sb[:, k, :], in_=xd[:, k, :])

            ones_e1 = consts.tile([E, 1], F32)
            nc.vector.memset(ones_e1[:], 1.0)
            ones_1e = consts.tile([1, E], F32)
            nc.vector.memset(ones_1e[:], 1.0)
            # Gated low-rank projection, fp16, rows 0..31 = wproj, row 32 = ones.
            wp_sb = consts.tile([ERA, TPC], F16)
            nc.vector.memset(wp_sb[ER : ER + 1, :], 1.0)

            # ---------- phase 1: proj + router matmuls ----------
            pps = []
            for t in range(NS_):
                tsl = slice(t * NSLAB, (t + 1) * NSLAB)
                # rows 0..31: proj^T (er, t); rows 32..35: router logits (e, t)
                pp = psum_proj.tile(
                    [ER + E, NSLAB], F32, tag=f"pp{t}", name=f"pp{t}"
                )
                for k in range(KT):
                    nc.tensor.matmul(
                        pp[:],
                        art_sb[:, k, :],
                        x_sb[:, k, tsl],
                        start=(k == 0),
                        stop=(k == KT - 1),
                    )
                pps.append(pp)

            def gating(t):
                # softmax over the 4 expert rows (no max-sub: |logit| < ~8),
                # partition reductions/broadcasts done with tiny PE matmuls
                tsl = slice(t * NSLAB, (t + 1) * NSLAB)
                pp = pps[t]
                e_sb = small.tile([E, NSLAB], F32, tag="e", name="e_sb")
                nc.scalar.activation(
                    e_sb[:], pp[ER : ER + E, :], mybir.ActivationFunctionType.Exp
                )
                s_ps = psum_proj.tile([1, NSLAB], F32, tag="gat", name="s_ps")
                nc.tensor.matmul(s_ps[:], ones_e1[:], e_sb[:])  # sum_e exp
                r_sb = small.tile([1, NSLAB], F32, tag="r", name="r_sb")
                nc.vector.reciprocal(r_sb[:], s_ps[:])
                r4_ps = psum_proj.tile([E, NSLAB], F32, tag="gat", name="r4_ps")
                nc.tensor.matmul(r4_ps[:], ones_1e[:], r_sb[:])  # bcast to 4 rows
                g4_sb = small.tile([E, NSLAB], F32, tag="g4", name="g4_sb")
                nc.vector.tensor_mul(g4_sb[:], e_sb[:], r4_ps[:])
                # (SCALE * gate)[er, t] via 0/1*SCALE selection matmul
                g32_ps = psum_proj.tile([ER, NSLAB], F32, tag="gat", name="g32_ps")
                nc.tensor.matmul(g32_ps[:], sel_sb[:], g4_sb[:])
                # walrus: tensor_tensor may read at most one operand from PSUM
                g32_sb = small.tile([ER, NSLAB], F32, tag="g32s", name="g32_sb")
                nc.vector.tensor_copy(g32_sb[:], g32_ps[:])
                nc.vector.tensor_mul(wp_sb[0:ER, tsl], pp[0:ER, :], g32_sb[:])

            # ---------- phase 2: base matmul + lora + bias ----------
            def base_kloop(ot):
                if ot not in w_tiles:
                    load_w(ot)
                pots = [
                    psum_base.tile([P, NSLAB], F32, tag=f"po{t}", name=f"po{t}")
                    for t in range(NS_)
                ]
                for k in range(KT):
                    for t in range(NS_):
                        nc.tensor.matmul(
                            pots[t][:],
                            w_tiles[ot][:, k, :],
                            x_sb[:, k, t * NSLAB : (t + 1) * NSLAB],
                            start=(k == 0),
                            stop=False,
                        )
                return pots

            def base_tail(ot, pots):
                osl = slice(ot * P, (ot + 1) * P)
                for t in range(NS_):
                    nc.tensor.matmul(
                        pots[t][:],
                        bt_sb[:, osl],
                        wp_sb[:, t * NSLAB : (t + 1) * NSLAB],
                        start=False,
                        stop=True,
                    )
                o_sb = opool.tile([P, TPC], F32, tag="o", name="o_sb")
                for t in range(NS_):
                    nc.vector.tensor_copy(
                        o_sb[:, t * NSLAB : (t + 1) * NSLAB], pots[t][:]
                    )
                nc.sync.dma_start(out=od[osl, :], in_=o_sb[:])
                del w_tiles[ot]

            for rep in range(reps):
                if rep == 0:
                    # o-tile 0's k-loop keeps the PE busy during the gating chain
                    pots0 = base_kloop(0)
                    for t in range(NS_):
                        gating(t)
                    base_tail(0, pots0)
                    start_ot = 1
                else:
                    start_ot = 0
                for ot in range(start_ot, OT):
                    pots = base_kloop(ot)
                    base_tail(ot, pots)

    nc.compile()
    _dedupe_ldweights(nc)
    return nc


def get_nc():
    global _NC
    if _NC is None:
        _NC = build_nc()
    return _NC


def _prep_shared(W, b, A, Bm, Wr):
    # W (OUT, IN) -> wd[ot, p, k, o] = W[ot*128+o, k*128+p], fp16, contiguous
    wd = np.ascontiguousarray(
        W.reshape(OT, P, KT, P).transpose(0, 3, 2, 1).astype(np.float16)
    )
    # [A (E,R,IN) flattened; Wr (E,IN)] -> art[p, k, j] = AR[j, k*128+p]
    ar = np.concatenate([A.reshape(ER, IN), Wr], axis=0)  # (36, IN)
    artd = np.ascontiguousarray(
        ar.T.reshape(KT, P, ER + E).transpose(1, 0, 2).astype(np.float16)
    )
    # Bcat rows er = Bm[e,:,r]; row 32 = bias
    bt = np.concatenate([Bm.transpose(0, 2, 1).reshape(ER, OUT), b[None, :]], axis=0)
    btd = np.ascontiguousarray(bt.astype(np.float16))
    sel = np.zeros((E, ER), np.float32)
    for e in range(E):
        sel[e, e * R : (e + 1) * R] = SCALE
    return wd, artd, btd, sel


def _prep_x_shard(xt, c):
    xs = xt[c * TPC : (c + 1) * TPC]  # (TPC, IN)
    return np.ascontiguousarray(
        xs.T.reshape(KT, P, TPC).transpose(1, 0, 2).astype(np.float16)
    )


def make_in_maps(x, W, b, A, Bm, Wr):
    xt = np.asarray(x, np.float32).reshape(TOK, IN)
    wd, artd, btd, sel = _prep_shared(
        np.asarray(W, np.float32),
        np.asarray(b, np.float32),
        np.asarray(A, np.float32),
        np.asarray(Bm, np.float32),
        np.asarray(Wr, np.float32),
    )
    return [
        {
            "xd": _prep_x_shard(xt, c),
            "wd": wd,
            "artd": artd,
            "btd": btd,
            "seld": sel,
        }
        for c in range(N_CORES)
    ]


def gather_out(results):
    # per-core od is (OUT, TPC); tokens are sharded contiguously
    return np.concatenate([r["od"].T for r in results], axis=0).reshape(B, S, OUT)


def kernel(x, W, b, A, Bm, Wr, _trace=False):
    nc = get_nc()
    in_maps = make_in_maps(x, W, b, A, Bm, Wr)
    res = bass_utils.run_bass_kernel_spmd(
        nc, in_maps, core_ids=list(range(N_CORES)), trace=_trace
    )
    out = gather_out(res.results)
    if _trace:
        return out, res
    return out



# revision 7
# speedup vs baseline: 1.2576x; 1.2576x over previous
"""MoE QLoRA linear kernel for Trainium2 (8 NeuronCores, data-parallel over tokens).

Computes, for x:(B,S,IN) f32:
    base  = x @ W.T + b
    gates = softmax(x @ Wr.T)                       # (tok, E)
    proj  = x @ A[e].T                              # (tok, E, R)
    out   = base + sum_e SCALE * gates[...,e] * (proj[...,e,:] @ Bm[e].T)

Key algebraic fold: the gated expert mix is a single rank-(E*R) matmul:
    wproj[t, er] = SCALE * gates[t, e] * proj[t, er]          (er = e*R+r)
    lora[t, o]   = sum_er wproj[t, er] * Bcat[er, o]          (Bcat[er,o] = Bm[e,o,r])
and the bias b is folded in as an extra contraction row (wproj row of ones,
Bcat row = b), so base+lora+bias all accumulate in one PSUM group on the PE.

Per-core kernel (1024 tokens), everything oriented (feature-partition, token-free):
  phase 1: PSUM(36,512) = [A;Wr]^T-stationary matmuls over 32 k-tiles ->
           proj rows 0..31, router logits rows 32..35; softmax via exp +
           PE ones-matmul partition reductions/broadcasts; wproj written fp16.
           The o-tile-0 base k-loop is emitted between the proj matmuls and
           the gating chain so the PE stays busy while ACT/DVE run softmax.
  phase 2: for each of 32 o-tiles: out(128o, t) = W-tile-stationary matmul
           over 32 k-tiles + one lora matmul (k=33) accumulated into PSUM,
           copy to SBUF, DMA out as (OUT, tok); host transposes back.

All matmul inputs are fp16 (host-cast; PE runs fp16 at full bf16 rate,
fp32 PSUM accumulation). Host pre-tiles all layouts so every DMA is
contiguous and the kernel needs zero on-chip transposes.

Perf note: this toolchain's walrus runs with --enable-ldw-opt=false (=true
crashes codegen), so every matmul gets its own LDWEIGHTS (~53ns each,
~115us/core) on top of the ~485us cost-model prediction. Matmul count
(2182) is at the hardware floor (m<=128, n<=512/PSUM bank, k<=128), so no
tiling change reduces it; the timeline trace shows the PE sequencer
saturated end-to-end apart from ~11us startup DMA and ~5us drain.

Landed optimizations beyond the original baseline:
1. LDWEIGHTS dedup surgery: after nc.compile(), _dedupe_ldweights deletes
   1066 redundant consecutive InstLdweights (the repeat load of each
   same-stationary (o-tile, k) slab pair; wait/update-free so the
   semaphore graph is untouched).  A matmul whose stationary key does not
   match the last load resets the tracker: the tile scheduler interleaves
   the fp32 *self-loading* gating matmuls into the o-tile-1 k-loop, and a
   self-loading matmul clobbers the PE array (deleting across one of those
   corrupts exactly that (o-tile, slab) block — hardware-verified).
2. Startup DMA reorder: the first 4 x k-tiles are queued ahead of the
   ~2.5MB of W/bt prefetch so phase 1 is never input-starved (~5us).

Explored and rejected (see transcript): pair-hybrid sharding (2 cores
share 2048 tokens, each takes half the output features) with a 2-core
AllGather of the gated projection.  It halves the distinct W stationaries
(512 vs 1024) and was numerically correct on hardware (rel err 3.2e-3,
collective included), but paired interleaved timing showed no advantage
(median +27us): the theoretical 28us/rep LDWEIGHTS saving did not
materialize on hardware, and the restructure costs ~15us of model-level
schedule/pacing stalls.  fp8 is also a dead end for the base matmul: e4m3
quantization of W alone costs ~2.4e-2 relative error (at the 2e-2 gate),
and a hi+lo fp8 split at DoubleRow rate (0.5 cyc/row) exactly cancels.
"""

import numpy as np

import concourse.bass as bass
import concourse.tile as tile
from concourse import bacc, mybir
from concourse import bass_utils

# Problem shape (hardcoded; kernel.py must be self-contained)
B, S, IN, OUT, E, R = 4, 2048, 4096, 4096, 4, 8
SCALE = 16.0 / 8.0
N_CORES = 8
TOK = B * S                  # 8192 tokens
TPC = TOK // N_CORES         # 1024 tokens per core
P = 128                      # partitions
KT = IN // P                 # 32 k-tiles (contraction)
OT = OUT // P                # 32 output tiles
NSLAB = 512                  # moving-operand free size (PSUM bank = 512 f32)
NS = TPC // NSLAB            # 2 token slabs per core
ER = E * R                   # 32 low-rank rows
ERA = ER + 1                 # +1 ones row (bias fold)

F16 = mybir.dt.float16
F32 = mybir.dt.float32

_NC = None


def _dedupe_ldweights(nc):
    """Delete redundant consecutive InstLdweights from the PE streams.

    With --enable-ldw-opt=false every matmul gets its own ~53ns LDWEIGHTS;
    when consecutive matmuls share a stationary (the two token-slab matmuls
    of each (o-tile, k) pair) the repeat loads are pure PE stalls.  A
    deleted load's matmuls pair with the previous (identical) load.  Only
    wait/update-free loads whose key matches the immediately preceding PE
    ldweights are deleted; any other PE instruction type resets the match.

    CRITICAL: a matmul with an fp32 stationary is self-loading and clobbers
    the PE array (the tile scheduler interleaves the f32 gating matmuls into
    the o-tile-1 k-loop), so any matmul whose stationary key doesn't match
    the last load also resets the match.
    """

    def _wkey(ap):
        return (ap.memref, ap.offset, str(ap.ap), str(ap.dtype))

    ndel = 0
    for b in nc.main_func.blocks:
        il = b.instructions
        last_key = None
        dead_ids = set()
        for inst in il:
            if inst.engine != mybir.EngineType.PE:
                continue
            if isinstance(inst, mybir.InstLdweights):
                ap = inst.ins[0]
                key = (_wkey(ap), str(inst.tile_position), str(inst.tile_size),
                       str(inst.perf_mode), str(inst.is_transpose))
                if key == last_key and not inst.has_wait() and not inst.has_update():
                    dead_ids.add(id(inst))
                else:
                    last_key = key
            elif isinstance(inst, mybir.InstMatmult):
                if last_key is None or _wkey(inst.ins[1]) != last_key[0]:
                    last_key = None
        if dead_ids:
            keep = [i for i in il if id(i) not in dead_ids]
            il.clear()
            il.extend(keep)
            ndel += len(dead_ids)
    return ndel


def build_nc(reps=1, ns=NS):
    NS_ = ns
    nc = bacc.Bacc("TRN2", target_bir_lowering=False, debug=False)

    xd = nc.dram_tensor("xd", [P, KT, TPC], F16, kind="ExternalInput")
    wd = nc.dram_tensor("wd", [OT, P, KT, P], F16, kind="ExternalInput")
    artd = nc.dram_tensor("artd", [P, KT, ER + E], F16, kind="ExternalInput")
    btd = nc.dram_tensor("btd", [ERA, OUT], F16, kind="ExternalInput")
    seld = nc.dram_tensor("seld", [E, ER], F32, kind="ExternalInput")
    od = nc.dram_tensor("od", [OUT, TPC], F32, kind="ExternalOutput")

    with tile.TileContext(nc) as tc:
        with (
            tc.tile_pool(name="consts", bufs=1) as consts,
            tc.tile_pool(name="wpool", bufs=3) as wpool,
            tc.tile_pool(name="opool", bufs=3) as opool,
            tc.tile_pool(name="small", bufs=2) as small,
            tc.tile_pool(name="psum_proj", bufs=1, space="PSUM") as psum_proj,
            tc.tile_pool(name="psum_base", bufs=2, space="PSUM") as psum_base,
        ):
            art_sb = consts.tile([P, KT, ER + E], F16)
            nc.sync.dma_start(out=art_sb[:], in_=artd[:])

            # First x k-tiles right behind art so phase 1's k-loop can start
            # before the ~2.5 MB of W/bt prefetch drains (saves ~5us of PE
            # startup idle vs loading all consts first).
            x_sb = consts.tile([P, KT, TPC], F16)
            for k in range(4):
                nc.sync.dma_start(out=x_sb[:, k, :], in_=xd[:, k, :])

            bt_sb = consts.tile([ERA, OUT], F16)
            nc.sync.dma_start(out=bt_sb[:], in_=btd[:])
            sel_sb = consts.tile([E, ER], F32)
            nc.sync.dma_start(out=sel_sb[:], in_=seld[:])

            w_tiles = {}

            def load_w(ot):
                w_sb = wpool.tile([P, KT, P], F16, tag="w", name="w_sb")
                nc.sync.dma_start(out=w_sb[:], in_=wd[ot])
                w_tiles[ot] = w_sb

            # first two W tiles before the bulk x load: o-tile 0 can start
            # as soon as phase-1 finishes on the PE
            load_w(0)
            load_w(1)

            # Rest of the resident activations: x^T tiled (p, k, t), fp16, 8 MiB.
            for k in range(4, KT):
                nc.sync.dma_start(out=x_sb[:, k, :], in_=xd[:, k, :])

            ones_e1 = consts.tile([E, 1], F32)
            nc.vector.memset(ones_e1[:], 1.0)
            ones_1e = consts.tile([1, E], F32)
            nc.vector.memset(ones_1e[:], 1.0)
            # Gated low-rank projection, fp16, rows 0..31 = wproj, row 32 = ones.
            wp_sb = consts.tile([ERA, TPC], F16)
            nc.vector.memset(wp_sb[ER : ER + 1, :], 1.0)

            # ---------- phase 1: proj + router matmuls ----------
            pps = []
            for t in range(NS_):
                tsl = slice(t * NSLAB, (t + 1) * NSLAB)
                # rows 0..31: proj^T (er, t); rows 32..35: router logits (e, t)
                pp = psum_proj.tile(
                    [ER + E, NSLAB], F32, tag=f"pp{t}", name=f"pp{t}"
                )
                for k in range(KT):
                    nc.tensor.matmul(
                        pp[:],
                        art_sb[:, k, :],
                        x_sb[:, k, tsl],
                        start=(k == 0),
                        stop=(k == KT - 1),
                    )
                pps.append(pp)

            def gating(t):
                # softmax over the 4 expert rows (no max-sub: |logit| < ~8),
                # partition reductions/broadcasts done with tiny PE matmuls
                tsl = slice(t * NSLAB, (t + 1) * NSLAB)
                pp = pps[t]
                e_sb = small.tile([E, NSLAB], F32, tag="e", name="e_sb")
                nc.scalar.activation(
                    e_sb[:], pp[ER : ER + E, :], mybir.ActivationFunctionType.Exp
                )
                s_ps = psum_proj.tile([1, NSLAB], F32, tag="gat", name="s_ps")
                nc.tensor.matmul(s_ps[:], ones_e1[:], e_sb[:])  # sum_e exp
                r_sb = small.tile([1, NSLAB], F32, tag="r", name="r_sb")
                nc.vector.reciprocal(r_sb[:], s_ps[:])
                r4_ps = psum_proj.tile([E, NSLAB], F32, tag="gat", name="r4_ps")
                nc.tensor.matmul(r4_ps[:], ones_1e[:], r_sb[:])  # bcast to 4 rows
                g4_sb = small.tile([E, NSLAB], F32, tag="g4", name="g4_sb")
                nc.vector.tensor_mul(g4_sb[:], e_sb[:], r4_ps[:])
                # (SCALE * gate)[er, t] via 0/1*SCALE selection matmul
                g32_ps = psum_proj.tile([ER, NSLAB], F32, tag="gat", name="g32_ps")
                nc.tensor.matmul(g32_ps[:], sel_sb[:], g4_sb[:])
                # walrus: tensor_tensor may read at most one operand from PSUM
                g32_sb = small.tile([ER, NSLAB], F32, tag="g32s", name="g32_sb")
                nc.vector.tensor_copy(g32_sb[:], g32_ps[:])
                nc.vector.tensor_mul(wp_sb[0:ER, tsl], pp[0:ER, :], g32_sb[:])

            # ---------- phase 2: base matmul + lora + bias ----------
            def base_kloop(ot):
                if ot not in w_tiles:
                    load_w(ot)
                pots = [
                    psum_base.tile([P, NSLAB], F32, tag=f"po{t}", name=f"po{t}")
                    for t in range(NS_)
                ]
                for k in range(KT):
                    for t in range(NS_):
                        nc.tensor.matmul(
                            pots[t][:],
                            w_tiles[ot][:, k, :],
                            x_sb[:, k, t * NSLAB : (t + 1) * NSLAB],
                            start=(k == 0),
                            stop=False,
                        )
                return pots

            def base_tail(ot, pots):
                osl = slice(ot * P, (ot + 1) * P)
                for t in range(NS_):
                    nc.tensor.matmul(
                        pots[t][:],
                        bt_sb[:, osl],
                        wp_sb[:, t * NSLAB : (t + 1) * NSLAB],
                        start=False,
                        stop=True,
                    )
                o_sb = opool.tile([P, TPC], F32, tag="o", name="o_sb")
                for t in range(NS_):
                    nc.vector.tensor_copy(
                        o_sb[:, t * NSLAB : (t + 1) * NSLAB], pots[t][:]
                    )
                nc.sync.dma_start(out=od[osl, :], in_=o_sb[:])
                del w_tiles[ot]

            for rep in range(reps):
                if rep == 0:
                    # o-tile 0's k-loop keeps the PE busy during the gating chain
                    pots0 = base_kloop(0)
                    for t in range(NS_):
                        gating(t)
                    base_tail(0, pots0)
                    start_ot = 1
                else:
                    start_ot = 0
                for ot in range(start_ot, OT):
                    pots = base_kloop(ot)
                    base_tail(ot, pots)

    nc.compile()
    _dedupe_ldweights(nc)
    return nc


def get_nc():
    global _NC
    if _NC is None:
        _NC = build_nc()
    return _NC


def _prep_shared(W, b, A, Bm, Wr):
    # W (OUT, IN) -> wd[ot, p, k, o] = W[ot*128+o, k*128+p], fp16, contiguous
    wd = np.ascontiguousarray(
        W.reshape(OT, P, KT, P).transpose(0, 3, 2, 1).astype(np.float16)
    )
    # [A (E,R,IN) flattened; Wr (E,IN)] -> art[p, k, j] = AR[j, k*128+p]
    ar = np.concatenate([A.reshape(ER, IN), Wr], axis=0)  # (36, IN)
    artd = np.ascontiguousarray(
        ar.T.reshape(KT, P, ER + E).transpose(1, 0, 2).astype(np.float16)
    )
    # Bcat rows er = Bm[e,:,r]; row 32 = bias
    bt = np.concatenate([Bm.transpose(0, 2, 1).reshape(ER, OUT), b[None, :]], axis=0)
    btd = np.ascontiguousarray(bt.astype(np.float16))
    sel = np.zeros((E, ER), np.float32)
    for e in range(E):
        sel[e, e * R : (e + 1) * R] = SCALE
    return wd, artd, btd, sel


def _prep_x_shard(xt, c):
    xs = xt[c * TPC : (c + 1) * TPC]  # (TPC, IN)
    return np.ascontiguousarray(
        xs.T.reshape(KT, P, TPC).transpose(1, 0, 2).astype(np.float16)
    )


def make_in_maps(x, W, b, A, Bm, Wr):
    xt = np.asarray(x, np.float32).reshape(TOK, IN)
    wd, artd, btd, sel = _prep_shared(
        np.asarray(W, np.float32),
        np.asarray(b, np.float32),
        np.asarray(A, np.float32),
        np.asarray(Bm, np.float32),
        np.asarray(Wr, np.float32),
    )
    return [
        {
            "xd": _prep_x_shard(xt, c),
            "wd": wd,
            "artd": artd,
            "btd": btd,
            "seld": sel,
        }
        for c in range(N_CORES)
    ]


def gather_out(results):
    # per-core od is (OUT, TPC); tokens are sharded contiguously
    return np.concatenate([r["od"].T for r in results], axis=0).reshape(B, S, OUT)


def kernel(x, W, b, A, Bm, Wr, _trace=False):
    nc = get_nc()
    in_maps = make_in_maps(x, W, b, A, Bm, Wr)
    res = bass_utils.run_bass_kernel_spmd(
        nc, in_maps, core_ids=list(range(N_CORES)), trace=_trace
    )
    out = gather_out(res.results)
    if _trace:
        return out, res
    return out



# revision 8
# speedup vs baseline: 1.5484x; 1.2312x over previous
"""MoE QLoRA linear kernel for Trainium2 (8 NeuronCores, data-parallel over tokens).

Computes, for x:(B,S,IN) f32:
    base  = x @ W.T + b
    gates = softmax(x @ Wr.T)                       # (tok, E)
    proj  = x @ A[e].T                              # (tok, E, R)
    out   = base + sum_e SCALE * gates[...,e] * (proj[...,e,:] @ Bm[e].T)

Key algebraic fold: the gated expert mix is a single rank-(E*R) matmul:
    wproj[t, er] = SCALE * gates[t, e] * proj[t, er]          (er = e*R+r)
    lora[t, o]   = sum_er wproj[t, er] * Bcat[er, o]          (Bcat[er,o] = Bm[e,o,r])
and the bias b is folded in as an extra contraction row (wproj row of ones,
Bcat row = b), so base+lora+bias all accumulate in one PSUM group on the PE.

Per-core kernel (1024 tokens), everything oriented (feature-partition, token-free):
  phase 1: PSUM(36,512) = [A;Wr]^T-stationary matmuls over 32 k-tiles ->
           proj rows 0..31, router logits rows 32..35; softmax via exp +
           PE ones-matmul partition reductions/broadcasts; wproj written fp16.
           The o-tile-0 base k-loop is emitted between the proj matmuls and
           the gating chain so the PE stays busy while ACT/DVE run softmax.
  phase 2: for each of 32 o-tiles: out(128o, t) = W-tile-stationary matmul
           over 32 k-tiles + one lora matmul (k=33) accumulated into PSUM,
           copy to SBUF, DMA out as (OUT, tok); host transposes back.

All matmul inputs are fp16 (host-cast; PE runs fp16 at full bf16 rate,
fp32 PSUM accumulation). Host pre-tiles all layouts so every DMA is
contiguous and the kernel needs zero on-chip transposes.

Perf note: matmul count (2182) is at the hardware floor (m<=128,
n<=512/PSUM bank, k<=128), so no tiling change reduces it; the timeline
trace shows the PE sequencer saturated end-to-end apart from ~11us startup
DMA and ~5us drain.  Measured device time runs ~100-120us over the ~485us
cost-model prediction; a paired microbenchmark (8000 matmuls, same
stationary, 1 vs 8000 standalone InstLdweights: median delta 1.9 ns/LDW)
PROVED this gap is NOT LDWEIGHTS — standalone weight loads pipeline behind
the previous matmul's drain and are ~free.  The residual gap is
per-matmul issue/PSUM-group overhead and/or sustained-load clock, neither
addressable at this tiling.

Landed optimizations beyond the original baseline:
1. LDWEIGHTS dedup surgery: after nc.compile(), _dedupe_ldweights deletes
   1066 redundant consecutive InstLdweights (wait/update-free so the
   semaphore graph is untouched).  Measured effect is ~2us (see above),
   not the ~57us the baseline notes predicted; kept because it is
   hardware-validated correct and strictly non-negative.  A matmul whose
   stationary key does not match the last load resets the tracker: the
   tile scheduler interleaves the fp32 *self-loading* gating matmuls into
   the o-tile-1 k-loop, and a self-loading matmul clobbers the PE array
   (deleting across one of those corrupts exactly that (o-tile, slab)
   block — hardware-verified).
2. Startup DMA reorder: the first 4 x k-tiles are queued ahead of the
   ~2.5MB of W/bt prefetch so phase 1 is never input-starved (~5us).

Explored and rejected (see transcript): pair-hybrid sharding (2 cores
share 2048 tokens, each takes half the output features) with a 2-core
AllGather of the gated projection.  It halves the distinct W stationaries
(512 vs 1024) and was numerically correct on hardware (rel err 3.2e-3,
collective included), but paired interleaved timing showed no advantage
(median +27us): the theoretical 28us/rep LDWEIGHTS saving did not
materialize on hardware, and the restructure costs ~15us of model-level
schedule/pacing stalls.  fp8 is also a dead end for the base matmul: e4m3
quantization of W alone costs ~2.4e-2 relative error (at the 2e-2 gate),
and a hi+lo fp8 split at DoubleRow rate (0.5 cyc/row) exactly cancels.
"""

import numpy as np

import concourse.bass as bass
import concourse.tile as tile
from concourse import bacc, mybir
from concourse import bass_utils

# Problem shape (hardcoded; kernel.py must be self-contained)
B, S, IN, OUT, E, R = 4, 2048, 4096, 4096, 4, 8
SCALE = 16.0 / 8.0
N_CORES = 8
TOK = B * S                  # 8192 tokens
TPC = TOK // N_CORES         # 1024 tokens per core
P = 128                      # partitions
KT = IN // P                 # 32 k-tiles (contraction)
OT = OUT // P                # 32 output tiles
NSLAB = 512                  # moving-operand free size (PSUM bank = 512 f32)
NS = TPC // NSLAB            # 2 token slabs per core
ER = E * R                   # 32 low-rank rows
ERA = ER + 1                 # +1 ones row (bias fold)

F16 = mybir.dt.float16
F32 = mybir.dt.float32

_NC = None


def _dedupe_ldweights(nc):
    """Delete redundant consecutive InstLdweights from the PE streams.

    With --enable-ldw-opt=false every matmul gets its own ~53ns LDWEIGHTS;
    when consecutive matmuls share a stationary (the two token-slab matmuls
    of each (o-tile, k) pair) the repeat loads are pure PE stalls.  A
    deleted load's matmuls pair with the previous (identical) load.  Only
    wait/update-free loads whose key matches the immediately preceding PE
    ldweights are deleted; any other PE instruction type resets the match.

    CRITICAL: a matmul with an fp32 stationary is self-loading and clobbers
    the PE array (the tile scheduler interleaves the f32 gating matmuls into
    the o-tile-1 k-loop), so any matmul whose stationary key doesn't match
    the last load also resets the match.
    """

    def _wkey(ap):
        return (ap.memref, ap.offset, str(ap.ap), str(ap.dtype))

    ndel = 0
    for b in nc.main_func.blocks:
        il = b.instructions
        last_key = None
        dead_ids = set()
        for inst in il:
            if inst.engine != mybir.EngineType.PE:
                continue
            if isinstance(inst, mybir.InstLdweights):
                ap = inst.ins[0]
                key = (_wkey(ap), str(inst.tile_position), str(inst.tile_size),
                       str(inst.perf_mode), str(inst.is_transpose))
                if key == last_key and not inst.has_wait() and not inst.has_update():
                    dead_ids.add(id(inst))
                else:
                    last_key = key
            elif isinstance(inst, mybir.InstMatmult):
                if last_key is None or _wkey(inst.ins[1]) != last_key[0]:
                    last_key = None
        if dead_ids:
            keep = [i for i in il if id(i) not in dead_ids]
            il.clear()
            il.extend(keep)
            ndel += len(dead_ids)
    return ndel


def build_nc(reps=1, ns=NS):
    NS_ = ns
    nc = bacc.Bacc("TRN2", target_bir_lowering=False, debug=False)

    xd = nc.dram_tensor("xd", [P, KT, TPC], F16, kind="ExternalInput")
    wd = nc.dram_tensor("wd", [OT, P, KT, P], F16, kind="ExternalInput")
    artd = nc.dram_tensor("artd", [P, KT, ER + E], F16, kind="ExternalInput")
    btd = nc.dram_tensor("btd", [ERA, OUT], F16, kind="ExternalInput")
    seld = nc.dram_tensor("seld", [E, ER], F32, kind="ExternalInput")
    od = nc.dram_tensor("od", [OUT, TPC], F32, kind="ExternalOutput")

    with tile.TileContext(nc) as tc:
        with (
            tc.tile_pool(name="consts", bufs=1) as consts,
            tc.tile_pool(name="wpool", bufs=3) as wpool,
            tc.tile_pool(name="opool", bufs=3) as opool,
            tc.tile_pool(name="small", bufs=2) as small,
            tc.tile_pool(name="psum_proj", bufs=1, space="PSUM") as psum_proj,
            tc.tile_pool(name="psum_base", bufs=2, space="PSUM") as psum_base,
        ):
            art_sb = consts.tile([P, KT, ER + E], F16)
            nc.sync.dma_start(out=art_sb[:], in_=artd[:])

            # First x k-tiles right behind art so phase 1's k-loop can start
            # before the ~2.5 MB of W/bt prefetch drains (saves ~5us of PE
            # startup idle vs loading all consts first).
            x_sb = consts.tile([P, KT, TPC], F16)
            for k in range(4):
                nc.sync.dma_start(out=x_sb[:, k, :], in_=xd[:, k, :])

            bt_sb = consts.tile([ERA, OUT], F16)
            nc.sync.dma_start(out=bt_sb[:], in_=btd[:])
            sel_sb = consts.tile([E, ER], F32)
            nc.sync.dma_start(out=sel_sb[:], in_=seld[:])

            w_tiles = {}

            def load_w(ot):
                w_sb = wpool.tile([P, KT, P], F16, tag="w", name="w_sb")
                nc.sync.dma_start(out=w_sb[:], in_=wd[ot])
                w_tiles[ot] = w_sb

            # first two W tiles before the bulk x load: o-tile 0 can start
            # as soon as phase-1 finishes on the PE
            load_w(0)
            load_w(1)

            # Rest of the resident activations: x^T tiled (p, k, t), fp16, 8 MiB.
            for k in range(4, KT):
                nc.sync.dma_start(out=x_sb[:, k, :], in_=xd[:, k, :])

            ones_e1 = consts.tile([E, 1], F32)
            nc.vector.memset(ones_e1[:], 1.0)
            ones_1e = consts.tile([1, E], F32)
            nc.vector.memset(ones_1e[:], 1.0)
            # Gated low-rank projection, fp16, rows 0..31 = wproj, row 32 = ones.
            wp_sb = consts.tile([ERA, TPC], F16)
            nc.vector.memset(wp_sb[ER : ER + 1, :], 1.0)

            # ---------- phase 1: proj + router matmuls ----------
            pps = []
            for t in range(NS_):
                tsl = slice(t * NSLAB, (t + 1) * NSLAB)
                # rows 0..31: proj^T (er, t); rows 32..35: router logits (e, t)
                pp = psum_proj.tile(
                    [ER + E, NSLAB], F32, tag=f"pp{t}", name=f"pp{t}"
                )
                for k in range(KT):
                    nc.tensor.matmul(
                        pp[:],
                        art_sb[:, k, :],
                        x_sb[:, k, tsl],
                        start=(k == 0),
                        stop=(k == KT - 1),
                    )
                pps.append(pp)

            def gating(t):
                # softmax over the 4 expert rows (no max-sub: |logit| < ~8),
                # partition reductions/broadcasts done with tiny PE matmuls
                tsl = slice(t * NSLAB, (t + 1) * NSLAB)
                pp = pps[t]
                e_sb = small.tile([E, NSLAB], F32, tag="e", name="e_sb")
                nc.scalar.activation(
                    e_sb[:], pp[ER : ER + E, :], mybir.ActivationFunctionType.Exp
                )
                s_ps = psum_proj.tile([1, NSLAB], F32, tag="gat", name="s_ps")
                nc.tensor.matmul(s_ps[:], ones_e1[:], e_sb[:])  # sum_e exp
                r_sb = small.tile([1, NSLAB], F32, tag="r", name="r_sb")
                nc.vector.reciprocal(r_sb[:], s_ps[:])
                r4_ps = psum_proj.tile([E, NSLAB], F32, tag="gat", name="r4_ps")
                nc.tensor.matmul(r4_ps[:], ones_1e[:], r_sb[:])  # bcast to 4 rows
                g4_sb = small.tile([E, NSLAB], F32, tag="g4", name="g4_sb")
                nc.vector.tensor_mul(g4_sb[:], e_sb[:], r4_ps[:])
                # (SCALE * gate)[er, t] via 0/1*SCALE selection matmul
                g32_ps = psum_proj.tile([ER, NSLAB], F32, tag="gat", name="g32_ps")
                nc.tensor.matmul(g32_ps[:], sel_sb[:], g4_sb[:])
                # walrus: tensor_tensor may read at most one operand from PSUM
                g32_sb = small.tile([ER, NSLAB], F32, tag="g32s", name="g32_sb")
                nc.vector.tensor_copy(g32_sb[:], g32_ps[:])
                nc.vector.tensor_mul(wp_sb[0:ER, tsl], pp[0:ER, :], g32_sb[:])

            # ---------- phase 2: base matmul + lora + bias ----------
            def base_kloop(ot):
                if ot not in w_tiles:
                    load_w(ot)
                pots = [
                    psum_base.tile([P, NSLAB], F32, tag=f"po{t}", name=f"po{t}")
                    for t in range(NS_)
                ]
                for k in range(KT):
                    for t in range(NS_):
                        nc.tensor.matmul(
                            pots[t][:],
                            w_tiles[ot][:, k, :],
                            x_sb[:, k, t * NSLAB : (t + 1) * NSLAB],
                            start=(k == 0),
                            stop=False,
                        )
                return pots

            def base_tail(ot, pots):
                osl = slice(ot * P, (ot + 1) * P)
                for t in range(NS_):
                    nc.tensor.matmul(
                        pots[t][:],
                        bt_sb[:, osl],
                        wp_sb[:, t * NSLAB : (t + 1) * NSLAB],
                        start=False,
                        stop=True,
                    )
                o_sb = opool.tile([P, TPC], F32, tag="o", name="o_sb")
                for t in range(NS_):
                    nc.vector.tensor_copy(
                        o_sb[:, t * NSLAB : (t + 1) * NSLAB], pots[t][:]
                    )
                nc.sync.dma_start(out=od[osl, :], in_=o_sb[:])
                del w_tiles[ot]

            for rep in range(reps):
                if rep == 0:
                    # o-tile 0's k-loop keeps the PE busy during the gating chain
                    pots0 = base_kloop(0)
                    for t in range(NS_):
                        gating(t)
                    base_tail(0, pots0)
                    start_ot = 1
                else:
                    start_ot = 0
                for ot in range(start_ot, OT):
                    pots = base_kloop(ot)
                    base_tail(ot, pots)

    nc.compile()
    _dedupe_ldweights(nc)
    return nc


def get_nc():
    global _NC
    if _NC is None:
        _NC = build_nc()
    return _NC


def _prep_shared(W, b, A, Bm, Wr):
    # W (OUT, IN) -> wd[ot, p, k, o] = W[ot*128+o, k*128+p], fp16, contiguous
    wd = np.ascontiguousarray(
        W.reshape(OT, P, KT, P).transpose(0, 3, 2, 1).astype(np.float16)
    )
    # [A (E,R,IN) flattened; Wr (E,IN)] -> art[p, k, j] = AR[j, k*128+p]
    ar = np.concatenate([A.reshape(ER, IN), Wr], axis=0)  # (36, IN)
    artd = np.ascontiguousarray(
        ar.T.reshape(KT, P, ER + E).transpose(1, 0, 2).astype(np.float16)
    )
    # Bcat rows er = Bm[e,:,r]; row 32 = bias
    bt = np.concatenate([Bm.transpose(0, 2, 1).reshape(ER, OUT), b[None, :]], axis=0)
    btd = np.ascontiguousarray(bt.astype(np.float16))
    sel = np.zeros((E, ER), np.float32)
    for e in range(E):
        sel[e, e * R : (e + 1) * R] = SCALE
    return wd, artd, btd, sel


def _prep_x_shard(xt, c):
    xs = xt[c * TPC : (c + 1) * TPC]  # (TPC, IN)
    return np.ascontiguousarray(
        xs.T.reshape(KT, P, TPC).transpose(1, 0, 2).astype(np.float16)
    )


def make_in_maps(x, W, b, A, Bm, Wr):
    xt = np.asarray(x, np.float32).reshape(TOK, IN)
    wd, artd, btd, sel = _prep_shared(
        np.asarray(W, np.float32),
        np.asarray(b, np.float32),
        np.asarray(A, np.float32),
        np.asarray(Bm, np.float32),
        np.asarray(Wr, np.float32),
    )
    return [
        {
            "xd": _prep_x_shard(xt, c),
            "wd": wd,
            "artd": artd,
            "btd": btd,
            "seld": sel,
        }
        for c in range(N_CORES)
    ]


def gather_out(results):
    # per-core od is (OUT, TPC); tokens are sharded contiguously
    return np.concatenate([r["od"].T for r in results], axis=0).reshape(B, S, OUT)


def kernel(x, W, b, A, Bm, Wr, _trace=False):
    nc = get_nc()
    in_maps = make_in_maps(x, W, b, A, Bm, Wr)
    res = bass_utils.run_bass_kernel_spmd(
        nc, in_maps, core_ids=list(range(N_CORES)), trace=_trace
    )
    out = gather_out(res.results)
    if _trace:
        return out, res
    return out



# revision 10
# speedup vs baseline: 2.6310x; 1.6992x over previous
"""MoE QLoRA linear kernel for Trainium2 (8 NeuronCores, data-parallel over tokens).

Computes, for x:(B,S,IN) f32:
    base  = x @ W.T + b
    gates = softmax(x @ Wr.T)                       # (tok, E)
    proj  = x @ A[e].T                              # (tok, E, R)
    out   = base + sum_e SCALE * gates[...,e] * (proj[...,e,:] @ Bm[e].T)

Key algebraic fold: the gated expert mix is a single rank-(E*R) matmul:
    wproj[t, er] = SCALE * gates[t, e] * proj[t, er]          (er = e*R+r)
    lora[t, o]   = sum_er wproj[t, er] * Bcat[er, o]          (Bcat[er,o] = Bm[e,o,r])
and the bias b is folded in as an extra contraction row (wproj row of ones,
Bcat row = b), so base+lora+bias all accumulate in one PSUM group on the PE.

Per-core kernel (1024 tokens), everything oriented (feature-partition, token-free):
  phase 1: PSUM(36,512) = [A;Wr]^T-stationary matmuls over 32 k-tiles ->
           proj rows 0..31, router logits rows 32..35; softmax via exp +
           PE ones-matmul partition reductions/broadcasts; wproj written fp16.
           The o-tile-0 base k-loop is emitted between the proj matmuls and
           the gating chain so the PE stays busy while ACT/DVE run softmax.
  phase 2: for each of 32 o-tiles: out(128o, t) = W-tile-stationary matmul
           over 32 k-tiles + one lora matmul (k=33) accumulated into PSUM,
           copy to SBUF, DMA out as (OUT, tok); host transposes back.

All matmul inputs are fp16 (host-cast; PE runs fp16 at full bf16 rate,
fp32 PSUM accumulation). Host pre-tiles all layouts so every DMA is
contiguous and the kernel needs zero on-chip transposes.

Perf note: matmul count (2182) is at the hardware floor (m<=128,
n<=512/PSUM bank, k<=128), so no tiling change reduces it; the timeline
trace shows the PE sequencer saturated end-to-end apart from ~11us startup
DMA and ~5us drain.  Measured device time runs ~100-120us over the ~485us
cost-model prediction; a paired microbenchmark (8000 matmuls, same
stationary, 1 vs 8000 standalone InstLdweights: median delta 1.9 ns/LDW)
PROVED this gap is NOT LDWEIGHTS — standalone weight loads pipeline behind
the previous matmul's drain and are ~free.  The residual gap is
per-matmul issue/PSUM-group overhead and/or sustained-load clock, neither
addressable at this tiling.

Landed optimizations beyond the original baseline:
1. LDWEIGHTS dedup surgery: after nc.compile(), _dedupe_ldweights deletes
   1066 redundant consecutive InstLdweights (wait/update-free so the
   semaphore graph is untouched).  Measured effect is ~2us (see above),
   not the ~57us the baseline notes predicted; kept because it is
   hardware-validated correct and strictly non-negative.  A matmul whose
   stationary key does not match the last load resets the tracker: the
   tile scheduler interleaves the fp32 *self-loading* gating matmuls into
   the o-tile-1 k-loop, and a self-loading matmul clobbers the PE array
   (deleting across one of those corrupts exactly that (o-tile, slab)
   block — hardware-verified).
2. Startup DMA reorder: the first 4 x k-tiles are queued ahead of the
   ~2.5MB of W/bt prefetch so phase 1 is never input-starved (~5us).

Explored and rejected (see transcript): pair-hybrid sharding (2 cores
share 2048 tokens, each takes half the output features) with a 2-core
AllGather of the gated projection.  It halves the distinct W stationaries
(512 vs 1024) and was numerically correct on hardware (rel err 3.2e-3,
collective included), but paired interleaved timing showed no advantage
(median +27us): the theoretical 28us/rep LDWEIGHTS saving did not
materialize on hardware, and the restructure costs ~15us of model-level
schedule/pacing stalls.  fp8 is also a dead end for the base matmul: e4m3
quantization of W alone costs ~2.4e-2 relative error (at the 2e-2 gate),
and a hi+lo fp8 split at DoubleRow rate (0.5 cyc/row) exactly cancels.
"""

import numpy as np

import concourse.bass as bass
import concourse.tile as tile
from concourse import bacc, mybir
from concourse import bass_utils

# Problem shape (hardcoded; kernel.py must be self-contained)
B, S, IN, OUT, E, R = 4, 2048, 4096, 4096, 4, 8
SCALE = 16.0 / 8.0
N_CORES = 8
TOK = B * S                  # 8192 tokens
TPC = TOK // N_CORES         # 1024 tokens per core
P = 128                      # partitions
KT = IN // P                 # 32 k-tiles (contraction)
OT = OUT // P                # 32 output tiles
NSLAB = 512                  # moving-operand free size (PSUM bank = 512 f32)
NS = TPC // NSLAB            # 2 token slabs per core
ER = E * R                   # 32 low-rank rows
ERA = ER + 1                 # +1 ones row (bias fold)

F16 = mybir.dt.float16
F32 = mybir.dt.float32

_NC = None


def _dedupe_ldweights(nc):
    """Delete redundant consecutive InstLdweights from the PE streams.

    With --enable-ldw-opt=false every matmul gets its own ~53ns LDWEIGHTS;
    when consecutive matmuls share a stationary (the two token-slab matmuls
    of each (o-tile, k) pair) the repeat loads are pure PE stalls.  A
    deleted load's matmuls pair with the previous (identical) load.  Only
    wait/update-free loads whose key matches the immediately preceding PE
    ldweights are deleted; any other PE instruction type resets the match.

    CRITICAL: a matmul with an fp32 stationary is self-loading and clobbers
    the PE array (the tile scheduler interleaves the f32 gating matmuls into
    the o-tile-1 k-loop), so any matmul whose stationary key doesn't match
    the last load also resets the match.
    """

    def _wkey(ap):
        return (ap.memref, ap.offset, str(ap.ap), str(ap.dtype))

    ndel = 0
    for b in nc.main_func.blocks:
        il = b.instructions
        last_key = None
        dead_ids = set()
        for inst in il:
            if inst.engine != mybir.EngineType.PE:
                continue
            if isinstance(inst, mybir.InstLdweights):
                ap = inst.ins[0]
                key = (_wkey(ap), str(inst.tile_position), str(inst.tile_size),
                       str(inst.perf_mode), str(inst.is_transpose))
                if key == last_key and not inst.has_wait() and not inst.has_update():
                    dead_ids.add(id(inst))
                else:
                    last_key = key
            elif isinstance(inst, mybir.InstMatmult):
                if last_key is None or _wkey(inst.ins[1]) != last_key[0]:
                    last_key = None
        if dead_ids:
            keep = [i for i in il if id(i) not in dead_ids]
            il.clear()
            il.extend(keep)
            ndel += len(dead_ids)
    return ndel


def build_nc(reps=1, ns=NS):
    NS_ = ns
    nc = bacc.Bacc("TRN2", target_bir_lowering=False, debug=False)

    xd = nc.dram_tensor("xd", [P, KT, TPC], F16, kind="ExternalInput")
    wd = nc.dram_tensor("wd", [OT, P, KT, P], F16, kind="ExternalInput")
    artd = nc.dram_tensor("artd", [P, KT, ER + E], F16, kind="ExternalInput")
    btd = nc.dram_tensor("btd", [ERA, OUT], F16, kind="ExternalInput")
    seld = nc.dram_tensor("seld", [E, ER], F32, kind="ExternalInput")
    od = nc.dram_tensor("od", [OUT, TPC], F32, kind="ExternalOutput")

    with tile.TileContext(nc) as tc:
        with (
            tc.tile_pool(name="consts", bufs=1) as consts,
            tc.tile_pool(name="wpool", bufs=3) as wpool,
            tc.tile_pool(name="opool", bufs=3) as opool,
            tc.tile_pool(name="small", bufs=2) as small,
            tc.tile_pool(name="psum_proj", bufs=1, space="PSUM") as psum_proj,
            tc.tile_pool(name="psum_base", bufs=2, space="PSUM") as psum_base,
        ):
            art_sb = consts.tile([P, KT, ER + E], F16)
            nc.sync.dma_start(out=art_sb[:], in_=artd[:])

            # First x k-tiles right behind art so phase 1's k-loop can start
            # before the ~2.5 MB of W/bt prefetch drains (saves ~5us of PE
            # startup idle vs loading all consts first).
            x_sb = consts.tile([P, KT, TPC], F16)
            for k in range(4):
                nc.sync.dma_start(out=x_sb[:, k, :], in_=xd[:, k, :])

            bt_sb = consts.tile([ERA, OUT], F16)
            nc.sync.dma_start(out=bt_sb[:], in_=btd[:])
            sel_sb = consts.tile([E, ER], F32)
            nc.sync.dma_start(out=sel_sb[:], in_=seld[:])

            w_tiles = {}

            def load_w(ot):
                w_sb = wpool.tile([P, KT, P], F16, tag="w", name="w_sb")
                nc.sync.dma_start(out=w_sb[:], in_=wd[ot])
                w_tiles[ot] = w_sb

            # first two W tiles before the bulk x load: o-tile 0 can start
            # as soon as phase-1 finishes on the PE
            load_w(0)
            load_w(1)

            # Rest of the resident activations: x^T tiled (p, k, t), fp16, 8 MiB.
            for k in range(4, KT):
                nc.sync.dma_start(out=x_sb[:, k, :], in_=xd[:, k, :])

            ones_e1 = consts.tile([E, 1], F32)
            nc.vector.memset(ones_e1[:], 1.0)
            ones_1e = consts.tile([1, E], F32)
            nc.vector.memset(ones_1e[:], 1.0)
            # Gated low-rank projection, fp16, rows 0..31 = wproj, row 32 = ones.
            wp_sb = consts.tile([ERA, TPC], F16)
            nc.vector.memset(wp_sb[ER : ER + 1, :], 1.0)

            # ---------- phase 1: proj + router matmuls ----------
            pps = []
            for t in range(NS_):
                tsl = slice(t * NSLAB, (t + 1) * NSLAB)
                # rows 0..31: proj^T (er, t); rows 32..35: router logits (e, t)
                pp = psum_proj.tile(
                    [ER + E, NSLAB], F32, tag=f"pp{t}", name=f"pp{t}"
                )
                for k in range(KT):
                    nc.tensor.matmul(
                        pp[:],
                        art_sb[:, k, :],
                        x_sb[:, k, tsl],
                        start=(k == 0),
                        stop=(k == KT - 1),
                    )
                pps.append(pp)

            def gating(t):
                # softmax over the 4 expert rows (no max-sub: |logit| < ~8),
                # partition reductions/broadcasts done with tiny PE matmuls
                tsl = slice(t * NSLAB, (t + 1) * NSLAB)
                pp = pps[t]
                e_sb = small.tile([E, NSLAB], F32, tag="e", name="e_sb")
                nc.scalar.activation(
                    e_sb[:], pp[ER : ER + E, :], mybir.ActivationFunctionType.Exp
                )
                s_ps = psum_proj.tile([1, NSLAB], F32, tag="gat", name="s_ps")
                nc.tensor.matmul(s_ps[:], ones_e1[:], e_sb[:])  # sum_e exp
                r_sb = small.tile([1, NSLAB], F32, tag="r", name="r_sb")
                nc.vector.reciprocal(r_sb[:], s_ps[:])
                r4_ps = psum_proj.tile([E, NSLAB], F32, tag="gat", name="r4_ps")
                nc.tensor.matmul(r4_ps[:], ones_1e[:], r_sb[:])  # bcast to 4 rows
                g4_sb = small.tile([E, NSLAB], F32, tag="g4", name="g4_sb")
                nc.vector.tensor_mul(g4_sb[:], e_sb[:], r4_ps[:])
                # (SCALE * gate)[er, t] via 0/1*SCALE selection matmul
                g32_ps = psum_proj.tile([ER, NSLAB], F32, tag="gat", name="g32_ps")
                nc.tensor.matmul(g32_ps[:], sel_sb[:], g4_sb[:])
                # walrus: tensor_tensor may read at most one operand from PSUM
                g32_sb = small.tile([ER, NSLAB], F32, tag="g32s", name="g32_sb")
                nc.vector.tensor_copy(g32_sb[:], g32_ps[:])
                nc.vector.tensor_mul(wp_sb[0:ER, tsl], pp[0:ER, :], g32_sb[:])

            # ---------- phase 2: base matmul + lora + bias ----------
            def base_kloop(ot):
                if ot not in w_tiles:
                    load_w(ot)
                pots = [
                    psum_base.tile([P, NSLAB], F32, tag=f"po{t}", name=f"po{t}")
                    for t in range(NS_)
                ]
                for k in range(KT):
                    for t in range(NS_):
                        nc.tensor.matmul(
                            pots[t][:],
                            w_tiles[ot][:, k, :],
                            x_sb[:, k, t * NSLAB : (t + 1) * NSLAB],
                            start=(k == 0),
                            stop=False,
                        )
                return pots

            def base_tail(ot, pots):
                osl = slice(ot * P, (ot + 1) * P)
                for t in range(NS_):
                    nc.tensor.matmul(
                        pots[t][:],
                        bt_sb[:, osl],
                        wp_sb[:, t * NSLAB : (t + 1) * NSLAB],
                        start=False,
                        stop=True,
                    )
                o_sb = opool.tile([P, TPC], F32, tag="o", name="o_sb")
                for t in range(NS_):
                    nc.vector.tensor_copy(
                        o_sb[:, t * NSLAB : (t + 1) * NSLAB], pots[t][:]
                    )
                nc.sync.dma_start(out=od[osl, :], in_=o_sb[:])
                del w_tiles[ot]

            for rep in range(reps):
                if rep == 0:
                    # o-tile 0's k-loop keeps the PE busy during the gating chain
                    pots0 = base_kloop(0)
                    for t in range(NS_):
                        gating(t)
                    base_tail(0, pots0)
                    start_ot = 1
                else:
                    start_ot = 0
                for ot in range(start_ot, OT):
                    pots = base_kloop(ot)
                    base_tail(ot, pots)

    nc.compile()
    _dedupe_ldweights(nc)
    return nc


def get_nc():
    global _NC
    if _NC is None:
        _NC = build_nc()
    return _NC


def _prep_shared(W, b, A, Bm, Wr):
    # W (OUT, IN) -> wd[ot, p, k, o] = W[ot*128+o, k*128+p], fp16, contiguous
    wd = np.ascontiguousarray(
        W.reshape(OT, P, KT, P).transpose(0, 3, 2, 1).astype(np.float16)
    )
    # [A (E,R,IN) flattened; Wr (E,IN)] -> art[p, k, j] = AR[j, k*128+p]
    ar = np.concatenate([A.reshape(ER, IN), Wr], axis=0)  # (36, IN)
    artd = np.ascontiguousarray(
        ar.T.reshape(KT, P, ER + E).transpose(1, 0, 2).astype(np.float16)
    )
    # Bcat rows er = Bm[e,:,r]; row 32 = bias
    bt = np.concatenate([Bm.transpose(0, 2, 1).reshape(ER, OUT), b[None, :]], axis=0)
    btd = np.ascontiguousarray(bt.astype(np.float16))
    sel = np.zeros((E, ER), np.float32)
    for e in range(E):
        sel[e, e * R : (e + 1) * R] = SCALE
    return wd, artd, btd, sel


def _prep_x_shard(xt, c):
    xs = xt[c * TPC : (c + 1) * TPC]  # (TPC, IN)
    return np.ascontiguousarray(
        xs.T.reshape(KT, P, TPC).transpose(1, 0, 2).astype(np.float16)
    )


def make_in_maps(x, W, b, A, Bm, Wr):
    xt = np.asarray(x, np.float32).reshape(TOK, IN)
    wd, artd, btd, sel = _prep_shared(
        np.asarray(W, np.float32),
        np.asarray(b, np.float32),
        np.asarray(A, np.float32),
        np.asarray(Bm, np.float32),
        np.asarray(Wr, np.float32),
    )
    return [
        {
            "xd": _prep_x_shard(xt, c),
            "wd": wd,
            "artd": artd,
            "btd": btd,
            "seld": sel,
        }
        for c in range(N_CORES)
    ]


def gather_out(results):
    # per-core od is (OUT, TPC); tokens are sharded contiguously
    return np.concatenate([r["od"].T for r in results], axis=0).reshape(B, S, OUT)


def kernel(x, W, b, A, Bm, Wr, _trace=False):
    nc = get_nc()
    in_maps = make_in_maps(x, W, b, A, Bm, Wr)
    res = bass_utils.run_bass_kernel_spmd(
        nc, in_maps, core_ids=list(range(N_CORES)), trace=_trace
    )
    out = gather_out(res.results)
    if _trace:
        return out, res
    return out

